# revision 13
# baseline (speedup 1.0000x reference)
"""Trainium2 Bass kernel for nn_Conv2d_14147622273082.

Conv2d 3x3, stride 1, pad 1: x [8, 320, 64, 64] f32, hf8-coded weights
w_bits [320, 320, 3, 3] i32 (codes 0..255), bias codes b_bits [320] i32.
out = conv2d(x, hf8_decode(w_bits)) + hf8_decode(b_bits).

Strategy: data-parallel over batch (1 image per NeuronCore, 8 cores).
hf8 decode is a 256-entry LUT done host-side into fp16 (exact: every hf8
value is fp16-representable); weights are replicated to every core. The
image is uploaded pre-padded in fp16 ([66, 68] with zero borders), so no
on-device casts or border memsets are needed.

The conv is 9 shifted [Cin,Cout] x [Cin,pix] fp16 matmuls accumulated in
PSUM over 512-pixel tiles. Cin=320 splits into K-chunks (128, 128, 64);
tail kernel positions are packed in pairs on partitions (0:64, 64:128)
against pre-shifted tail images (xp2: +1 col, xb2: +1 row -2 col), so one
K=128 matmul computes two positions. The leftover solo position (pos 8,
K=64) is row-tiled: even pixel tiles on PE rows 0:64, odd tiles on rows
64:128 (against a second unshifted tail image on partitions 64:128),
emitted adjacently so the two matmuls run concurrently.

Cout=320 = 128 + 128 + 64: the 64-wide output tail would waste half the
PE array columns, so it is computed as column-tiled concurrent matmul
pairs: two pixel tiles accumulate simultaneously in one PSUM bank, pixel
tile A on array columns 0:64 (tile_position (0,0), psum partitions 0:64)
and pixel tile B on columns 64:128 (tile_position (0,64), partitions
64:128) -- halving the tail chunk's PE time (measured ~2x).
"""

import numpy as np

import concourse.bass as bass
import concourse.tile as tile
from concourse import bacc, mybir
from concourse.bass_utils import run_bass_kernel_spmd

B, CIN, COUT, H, W = 8, 320, 320, 64, 64
PIX = H * W  # 4096
P = 128
HP, WP = H + 2, W + 4  # 66 x 68 padded image (2 cols pad keeps 4B align)
NT = 512  # pixels per psum tile = 8 rows of 64
RPT = NT // W  # 8
NPT = PIX // NT  # 8
# tail position pairing: pos = kh*3+kw; pairs (a, b) packed on partitions
# (0:64, 64:128). Pairs with flat-offset delta +1 use xp2 (lower half
# pre-shifted +1 col); the (2,3) pair has delta +66 and uses xb2.
TAIL_PAIRS = [(0, 1), (2, 3), (4, 5), (6, 7)]
N_ACC = 2 * 9 + len(TAIL_PAIRS) + 1  # 23

F16 = mybir.dt.float16
F32 = mybir.dt.float32
N_WARM = 44  # small matmuls covering the first-DMA latency


def _hf8_lut():
    bits = np.arange(256, dtype=np.int64)
    sign = np.where(((bits >> 7) & 1) == 1, -1.0, 1.0)
    exp = (bits >> 3) & 0xF
    man = (bits & 0x7).astype(np.float64)
    val = sign * np.where(
        exp == 0, 2.0 ** (1 - 14) * (man / 8.0), np.exp2(exp - 14.0) * (1 + man / 8.0)
    )
    return val


_LUT16 = _hf8_lut().astype(np.float16)  # exact in fp16
_LUT32 = _hf8_lut().astype(np.float32)


def build():
    from concourse.tile_rust import add_dep_helper

    nc = bacc.Bacc(
        "TRN2", target_bir_lowering=False, debug=False, enable_partition_id=False
    )
    xp_d = [
        nc.dram_tensor(f"xp{i}", [P, HP, WP], F16, kind="ExternalInput")
        for i in range(3)
    ]
    xb_d = nc.dram_tensor("xb2", [P, HP, WP], F16, kind="ExternalInput")
    xc_d = nc.dram_tensor("xc2", [64, HP, WP], F16, kind="ExternalInput")
    w0_d = nc.dram_tensor("w0", [P, 9, COUT], F16, kind="ExternalInput")
    w1_d = nc.dram_tensor("w1", [P, 9, COUT], F16, kind="ExternalInput")
    w2_d = nc.dram_tensor("w2", [P, 5, COUT], F16, kind="ExternalInput")
    bf_d = nc.dram_tensor("bf", [P, 4], F32, kind="ExternalInput")
    out_d = nc.dram_tensor("out", [COUT, PIX], F32, kind="ExternalOutput")

    with tile.TileContext(nc) as tc:
        with (
            tc.tile_pool(name="persist", bufs=1) as persist,
            tc.tile_pool(name="stage", bufs=1) as stage,
            tc.tile_pool(name="outsb", bufs=4) as outsb,
            tc.tile_pool(name="psum", bufs=1, space="PSUM") as pp,
        ):
            xt = [
                persist.tile([P, HP, WP], F16, tag=f"xt{i}", name=f"xt{i}")
                for i in range(5)
            ]
            wl0 = persist.tile([P, 9, COUT], F16, tag="wl0", name="wl0")
            wl1 = persist.tile([P, 9, COUT], F16, tag="wl1", name="wl1")
            wpair = persist.tile([P, 5, COUT], F16, tag="wpair", name="wpair")
            bf = persist.tile([P, 4], F32, tag="bf", name="bf")

            # ---- engine warmups (no data deps) ----
            wsrc = stage.tile([P, P], F16, tag="wsrc", name="wsrc")
            zsrc = stage.tile([P, 1], F32, tag="zsrc", name="zsrc")
            zo = stage.tile([P, 1], F32, tag="zo", name="zo")
            m0 = nc.gpsimd.memset(wsrc[:], 0.0)
            m1 = nc.gpsimd.memset(zsrc[:], 0.0)
            add_dep_helper(m1.ins, m0.ins, sync=False, reason="gpsimd order")
            act_warm = nc.scalar.activation(
                zo[:], zsrc[:], mybir.ActivationFunctionType.Identity, scale=1.0
            )

            # ---- input DMAs, deadline order, one in-order queue (a second
            # HWDGE queue on the scalar engine measured +45ns on every MM:
            # its in-flight transfers contend with PE SBUF streaming) ----
            nc.sync.dma_start(xt[0][:, 0:10], xp_d[0][:, 0:10])
            nc.sync.dma_start(wl0[:, 0:1], w0_d[:, 0:1])
            nc.sync.dma_start(wl0[:, 1:3], w0_d[:, 1:3])
            nc.sync.dma_start(xt[0][:, 10:26], xp_d[0][:, 10:26])
            nc.sync.dma_start(wl0[:, 3:6], w0_d[:, 3:6])
            nc.sync.dma_start(xt[0][:, 26:42], xp_d[0][:, 26:42])
            nc.sync.dma_start(wl0[:, 6:9], w0_d[:, 6:9])
            nc.sync.dma_start(xt[0][:, 42:66], xp_d[0][:, 42:66])
            nc.sync.dma_start(wl1[:], w1_d[:])
            nc.sync.dma_start(xt[1][:], xp_d[1][:])
            nc.sync.dma_start(wpair[:], w2_d[:])
            nc.sync.dma_start(xt[2][:], xp_d[2][:])
            nc.sync.dma_start(xt[3][:], xb_d[:])
            nc.sync.dma_start(xt[4][64:P], xc_d[:])
            nc.sync.dma_start(bf[:], bf_d[:])
            warm_ps = pp.tile([P, NT], F32, tag="acc7", name="warm_ps")
            for _ in range(N_WARM):
                nc.tensor.matmul(
                    warm_ps[0:64, 0:64], wsrc[:, 0:64], wsrc[:, 0:64],
                    start=True, stop=True,
                )

            # ---- matmul stream ----
            prev_act = {"a": act_warm}

            def epi(acc_t, bias_col, dsts):
                osb = outsb.tile([P, NT], F32, tag="osb", name="osb")
                a = nc.scalar.activation(
                    osb[:], acc_t,
                    mybir.ActivationFunctionType.Identity,
                    bias=bf[:, bias_col : bias_col + 1], scale=1.0,
                )
                add_dep_helper(
                    a.ins, prev_act["a"].ins, sync=False, reason="epi order"
                )
                prev_act["a"] = a
                for dst, rows in dsts:
                    nc.sync.dma_start(dst, osb[rows[0] : rows[1]])

            def full_chunk(ms, mi, staged):
                acc = [
                    pp.tile([P, NT], F32, tag=f"acc{t}", name=f"acc_{mi}_{t}")
                    for t in range(NPT)
                ]
                cnt = [0] * NPT

                def mm(lhsT, src, kh, kw, t, p0=0):
                    h0 = t * RPT
                    rhs = src[
                        p0 : p0 + lhsT.shape[0],
                        h0 + kh : h0 + kh + RPT,
                        kw + 1 : kw + 1 + W,
                    ]
                    nc.tensor.matmul(
                        acc[t][:P], lhsT, rhs,
                        start=(cnt[t] == 0), stop=(cnt[t] == N_ACC - 1),
                    )
                    cnt[t] += 1

                def pairs4(t):
                    for j, (pa, pb) in enumerate(TAIL_PAIRS):
                        src = xt[3] if (pa, pb) == (2, 3) else xt[2]
                        mm(wpair[:, j, ms : ms + P], src, pa // 3, pa % 3, t)

                def solo(t):
                    # row-tiled: even tiles on PE rows 0:64 (xp2 upper half),
                    # odd tiles on rows 64:128 (unshifted tail copy in xt4);
                    # tile_position auto-derives from the base partitions
                    if t % 2 == 0:
                        mm(wpair[0:64, 4, ms : ms + P], xt[2], 2, 2, t)
                    else:
                        mm(wpair[64:P, 4, ms : ms + P], xt[4], 2, 2, t, p0=64)

                if staged:
                    # ramp: only w positions 0-2 and image rows 0-9 resident
                    for pos in range(3):
                        mm(wl0[:, pos, ms : ms + P], xt[0], pos // 3, pos % 3, 0)
                    for t in range(1, 4):
                        for pos in range(3):
                            mm(wl0[:, pos, ms : ms + P], xt[0], pos // 3, pos % 3, t)
                    for pos in range(3, 9):
                        for t in range(4):
                            mm(wl0[:, pos, ms : ms + P], xt[0], pos // 3, pos % 3, t)
                    for pos in range(9):
                        for t in range(4, 8):
                            mm(wl0[:, pos, ms : ms + P], xt[0], pos // 3, pos % 3, t)
                    for pos in range(9):
                        for t in range(NPT):
                            mm(wl1[:, pos, ms : ms + P], xt[1], pos // 3, pos % 3, t)
                    for k in range(NPT // 2):
                        tA, tB = 2 * k, 2 * k + 1
                        pairs4(tA)
                        pairs4(tB)
                        solo(tA)
                        solo(tB)
                        for t in (tA, tB):
                            epi(acc[t][:P], mi,
                                [(out_d[ms : ms + P, t * NT : (t + 1) * NT], (0, P))])
                else:
                    # tile-pair-by-tile-pair so the two solos are adjacent
                    for k in range(NPT // 2):
                        tA, tB = 2 * k, 2 * k + 1
                        for t in (tA, tB):
                            for pos in range(9):
                                mm(wl0[:, pos, ms : ms + P], xt[0],
                                   pos // 3, pos % 3, t)
                            for pos in range(9):
                                mm(wl1[:, pos, ms : ms + P], xt[1],
                                   pos // 3, pos % 3, t)
                            pairs4(t)
                        solo(tA)
                        solo(tB)
                        for t in (tA, tB):
                            epi(acc[t][:P], mi,
                                [(out_d[ms : ms + P, t * NT : (t + 1) * NT], (0, P))])
                assert all(c == N_ACC for c in cnt), cnt

            full_chunk(0, 0, staged=True)

            # ---- co tail 256:320: column-tiled concurrent pixel-tile pairs
            # (run before the second full chunk so the kernel's final output
            # drain is one contiguous 256KB DMA, not two 131KB ones) ----
            cs = 256
            for k in range(4):
                tA, tB = 2 * k, 2 * k + 1
                pacc = pp.tile([P, NT], F32, tag=f"acc{k}", name=f"tacc{k}")
                cnt = [0]

                def pmm(lhsT, src, kh, kw, pacc=pacc, tA=tA, tB=tB, cnt=cnt):
                    first, last = cnt[0] == 0, cnt[0] == N_ACC - 1
                    np_ = lhsT.shape[0]
                    for col, t in ((0, tA), (64, tB)):
                        h0 = t * RPT
                        rhs = src[:np_, h0 + kh : h0 + kh + RPT, kw + 1 : kw + 1 + W]
                        nc.tensor.matmul(
                            pacc[col : col + 64], lhsT, rhs,
                            start=first, stop=last,
                            tile_position=(0, col),
                            skip_group_check=(col == 64),
                        )
                    cnt[0] += 1

                for pos in range(9):
                    pmm(wl0[:, pos, cs : cs + 64], xt[0], pos // 3, pos % 3)
                for pos in range(9):
                    pmm(wl1[:, pos, cs : cs + 64], xt[1], pos // 3, pos % 3)
                for j, (pa, pb) in enumerate(TAIL_PAIRS):
                    src = xt[3] if (pa, pb) == (2, 3) else xt[2]
                    pmm(wpair[:, j, cs : cs + 64], src, pa // 3, pa % 3)
                pmm(wpair[0:64, 4, cs : cs + 64], xt[2], 2, 2)
                assert cnt[0] == N_ACC
                epi(pacc[:P], 2,
                    [
                        (out_d[cs : cs + 64, tA * NT : (tA + 1) * NT], (0, 64)),
                        (out_d[cs : cs + 64, tB * NT : (tB + 1) * NT], (64, P)),
                    ])

            full_chunk(P, 1, staged=False)

    nc.compile()
    return nc


_NC_CACHE = None


def _get_nc():
    global _NC_CACHE
    if _NC_CACHE is None:
        _NC_CACHE = build()
    return _NC_CACHE


def _prep_in_maps(x, w_bits, b_bits):
    # host-side hf8 decode (exact fp16 LUT) + relayout [co,ci,kh,kw]->[ci,pos,co]
    w9 = _LUT16[w_bits.astype(np.uint8)].transpose(1, 2, 3, 0).reshape(CIN, 9, COUT)
    w0 = np.ascontiguousarray(w9[0:P])
    w1 = np.ascontiguousarray(w9[P : 2 * P])
    tail = w9[2 * P : CIN]  # [64, 9, 320]
    w2 = np.zeros((P, 5, COUT), np.float16)
    for j, (pa, pb) in enumerate(TAIL_PAIRS):
        w2[0:64, j] = tail[:, pa]
        w2[64:P, j] = tail[:, pb]
    w2[0:64, 4] = tail[:, 8]
    w2[64:P, 4] = tail[:, 8]
    b = _LUT32[b_bits.astype(np.uint8).reshape(COUT)]
    bfv = np.zeros((P, 4), np.float32)
    bfv[:, 0] = b[0:P]
    bfv[:, 1] = b[P : 2 * P]
    bfv[:, 2] = b[2 * P + (np.arange(P) % 64)]

    ins = []
    for i in range(B):
        xi = x[i].astype(np.float16)  # [320, 64, 64]
        xp = np.zeros((CIN, HP, WP), np.float16)
        xp[:, 1 : H + 1, 2 : W + 2] = xi
        xtail = xi[2 * P : CIN]  # [64, 64, 64]
        xp2 = np.zeros((P, HP, WP), np.float16)
        xp2[0:64] = xp[2 * P : CIN]
        xp2[64:P, 1 : H + 1, 1 : W + 1] = xtail  # shifted +1 col
        xb2 = np.zeros((P, HP, WP), np.float16)
        xb2[0:64] = xp[2 * P : CIN]
        xb2[64:P, 0:H, 4:WP] = xtail  # shifted +1 row, -2 col (flat +66)
        xc2 = np.ascontiguousarray(xp[2 * P : CIN])  # unshifted, for odd solos
        ins.append(
            {
                "xp0": np.ascontiguousarray(xp[0:P]),
                "xp1": np.ascontiguousarray(xp[P : 2 * P]),
                "xp2": xp2,
                "xb2": xb2,
                "xc2": xc2,
                "w0": w0,
                "w1": w1,
                "w2": w2,
                "bf": bfv,
            }
        )
    return ins


def kernel(x, w_bits, b_bits):
    nc = _get_nc()
    in_maps = _prep_in_maps(x, w_bits, b_bits)
    res = run_bass_kernel_spmd(nc, in_maps, core_ids=list(range(B)), trace=False)
    return np.stack(
        [res.results[i]["out"].reshape(COUT, H, W) for i in range(B)]
    ).astype(np.float32)


if __name__ == "__main__":
    rng = np.random.default_rng(0)
    x = rng.standard_normal((B, CIN, H, W)).astype(np.float32)
    w_bits = rng.integers(0, 256, (COUT, CIN, 3, 3)).astype(np.int32)
    b_bits = rng.integers(0, 256, (COUT,)).astype(np.int32)
    out = kernel(x, w_bits, b_bits)
    print("out", out.shape, out.dtype, float(np.abs(out).mean()))
